# revision 21
# baseline (speedup 1.0000x reference)
"""CRF loss kernel for Trainium2 (8 NeuronCores, data-parallel over batch).

v3 strategy — compacted emission stream + host-side pair histogram:
  - The reference loss = (sum_m E[b,s,tags] + sum_pm Tr[tag_prev,tag_cur]) / sum(m).
    All float arithmetic on emissions / transitions happens ON DEVICE; the
    host only marshals indices (mask compaction, tag packing, and the
    integer pair-count histogram C — all derived purely from tags+mask).
  - Emission score: host compacts the ~50% unmasked positions into a
    bf16 emission stream laid out as the exact SBUF image.  Per
    128-position tile the device builds a one-hot of the tags and
    accumulates Hm^T @ E into a [T,T] PSUM; the diagonal sums to the
    emission score.  One-hot builds are load-balanced over DVE
    (tensor_scalar is_equal in 4x mode, ~94ns), GPSIMD (~273ns) and ACT
    (relu(1-|tag-iota|), 2 activations, ~580ns).
  - Transition score: C[t1,t2] = #(valid pairs with tags t1->t2) is an
    integer histogram of index data, computed host-side; the device
    computes sum(C * transitions) with a fused multiply-reduce.
  - The emission stream lives wholly in SBUF (~34 KiB/partition), loaded
    with 4 large chunked DMAs (>=512B contiguous per descriptor, issued
    after the small tag/C/Tr tables so builds start immediately).
  - Per-core output is [128,2] per-partition partial sums; the 8-way
    combine, the mask count and the division happen host-side.
"""
import sys
import json

for p in ('/opt/trn_rl_repo', '/opt/trn_rl_repo/concourse'):
    if p not in sys.path:
        sys.path.insert(0, p)

import numpy as np
import ml_dtypes

BF16 = ml_dtypes.bfloat16
B, S, T = 512, 512, 128
NCORES = 8
BSH = B // NCORES              # 64 batch rows per core
NPOS = BSH * S                 # 32768 positions per core
PAD_TAG = 200.0                # out of [0,T) -> one-hot row is all zero


def _split_waits_json(bir_bytes: bytes, max_waits: int = 1) -> bytes:
    """This walrus build accepts at most ONE sync-wait per instruction;
    hoist extra waits onto single-wait NoOps inserted before the inst."""
    d = json.loads(bir_bytes)
    ctr = 0
    for f in d['functions']:
        for blk in f['blocks']:
            insts = blk.get('instructions')
            if not insts:
                continue
            out = []
            changed = False
            for ins in insts:
                si = ins.get('sync_info')
                if si and len(si.get('on_wait') or []) > max_waits:
                    waits = si['on_wait']
                    for w in waits[:-max_waits]:
                        ctr += 1
                        nop = {'engine': ins['engine'], 'ins': [], 'outs': [],
                               'name': f'wsplit-{ctr}', 'opcode': 'NoOp',
                               'sync_info': {'on_wait': [w], 'on_update': []}}
                        if 'debug' in ins:
                            nop['debug'] = ins['debug']
                        out.append(nop)
                    si['on_wait'] = waits[-max_waits:]
                    changed = True
                out.append(ins)
            if changed:
                blk['instructions'] = out
    return json.dumps(d).encode()


_patched = False


def _install_patch(bass_module):
    global _patched
    if _patched:
        return
    _patched = True
    orig = bass_module.Bass.to_json_bytes

    def patched(self):
        return _split_waits_json(orig(self))

    bass_module.Bass.to_json_bytes = patched


def _build(nte):
    """nte: number of 128-position tiles in the emission stream (mult of 4)."""
    import concourse.bass as bass
    import concourse.mybir as mybir
    import concourse.tile as tile
    from concourse.masks import make_identity
    _install_patch(bass)
    f32 = mybir.dt.float32
    bf16 = mybir.dt.bfloat16
    i32 = mybir.dt.int32
    Alu = mybir.AluOpType
    Af = mybir.ActivationFunctionType

    nbe = nte // 4                 # emission blocks of [128, 4, 128]

    nc = bass.Bass()
    em = nc.dram_tensor('em', [128, nte * T], bf16, kind='ExternalInput')
    tge = nc.dram_tensor('tge', [128, nte], f32, kind='ExternalInput')
    cm = nc.dram_tensor('cm', [T, T], f32, kind='ExternalInput')
    tr = nc.dram_tensor('tr', [T, T], f32, kind='ExternalInput')
    out = nc.dram_tensor('out', [128, 2], f32, kind='ExternalOutput')

    # ~6-block chunks so PE consumption keeps pace with transfers, with a
    # 1-block final chunk so the post-last-transfer tail is minimal
    chunks = []
    if nbe > 1:
        body = nbe - 1
        n_chunks = max(1, (body + 5) // 6)
        per_chunk = (body + n_chunks - 1) // n_chunks
        g0 = 0
        while g0 < body:
            g1 = min(body, g0 + per_chunk)
            chunks.append((g0, g1))
            g0 = g1
    chunks.append((nbe - 1, nbe))

    with tile.TileContext(nc) as tc:
        with tc.tile_pool(name='per', bufs=1) as per, \
             tc.tile_pool(name='hblk', bufs=34) as hblk, \
             tc.tile_pool(name='ps', bufs=1, space='PSUM') as psp:

            # ---- DMA order: first emission chunk, tag table, remaining
            # chunks, then the epilogue-only C/Tr tables.  The single
            # DMA-engines resource serializes transfers in this order. ----
            em_chunks = []

            def load_chunk(g0, g1):
                et = per.tile([128, (g1 - g0) * 4 * T], bf16, tag=f'emc{g0}')
                nc.sync.dma_start(out=et, in_=em[:, g0 * 4 * T:g1 * 4 * T])
                em_chunks.append((g0, g1, et))

            cm_sb = per.tile([128, 128], f32)
            nc.sync.dma_start(out=cm_sb, in_=cm[:, :])
            t_sb = per.tile([128, 128], f32)
            nc.sync.dma_start(out=t_sb, in_=tr[:, :])
            tge_sb = per.tile([128, nte], f32)
            nc.sync.dma_start(out=tge_sb, in_=tge[:, :])
            for (g0, g1) in chunks:
                load_chunk(g0, g1)

            # ---- constants ----
            iota_i = per.tile([128, 128], i32)
            nc.gpsimd.iota(iota_i, pattern=[[1, 128]], base=0, channel_multiplier=0)
            iota_b = per.tile([128, 128], bf16)
            nc.vector.tensor_copy(iota_b, iota_i)
            ident = per.tile([128, 128], f32)
            make_identity(nc, ident)

            ps_emit = psp.tile([128, 128], f32)
            red = per.tile([128, 2], f32)
            nc.vector.memset(red, 0.0)

            # ---- one-hot builds: greedy balance over DVE/GPSIMD/ACT ----
            load = {'d': 0.0, 'p': 0.0, 'a': 0.0}
            COST = {'d': 376.0, 'p': 1092.0, 'a': 2330.0}

            def build_tile4(dst, col_tile, k0):
                e = min(load, key=lambda x: load[x] + COST[x])
                load[e] += COST[e]
                if e == 'a':
                    tmp = hblk.tile([128, 4, 128], bf16, tag='atmp')
                    for j in range(4):
                        nc.scalar.activation(tmp[:, j, :], iota_b, Af.Abs,
                                             bias=col_tile[:, k0 + j:k0 + j + 1],
                                             scale=-1.0)
                        nc.scalar.activation(dst[:, j, :], tmp[:, j, :], Af.Relu,
                                             bias=1.0, scale=-1.0)
                    return
                eng = nc.vector if e == 'd' else nc.gpsimd
                for j in range(4):
                    eng.tensor_scalar(out=dst[:, j, :], in0=iota_b,
                                      scalar1=col_tile[:, k0 + j:k0 + j + 1],
                                      scalar2=None, op0=Alu.is_equal)

            n_mm = [0]
            for (g0, g1, et) in em_chunks:
                for g in range(g0, g1):
                    hm = hblk.tile([128, 4, 128], bf16, tag='hm')
                    build_tile4(hm, tge_sb, 4 * g)
                    for j in range(4):
                        first = n_mm[0] == 0
                        n_mm[0] += 1
                        last = n_mm[0] == nte
                        col = ((g - g0) * 4 + j) * T
                        nc.tensor.matmul(ps_emit, lhsT=hm[:, j, :],
                                         rhs=et[:, col:col + T],
                                         start=first, stop=last,
                                         skip_group_check=True)

            # ---- partial sums, one fused multiply+row-sum each: the
            # transition product on GPSIMD (idle, inputs ready mid-kernel),
            # the emission diag on DVE right after the last matmul ----
            scr2 = per.tile([128, 128], f32)
            nc.vector.scalar_tensor_tensor(out=scr2, in0=cm_sb, scalar=1.0,
                                           in1=t_sb, op0=Alu.mult, op1=Alu.mult,
                                           accum_out=red[:, 1:2])
            scr = per.tile([128, 128], f32)
            nc.vector.scalar_tensor_tensor(out=scr, in0=ps_emit, scalar=1.0,
                                           in1=ident, op0=Alu.mult, op1=Alu.mult,
                                           accum_out=red[:, 0:1])
            nc.sync.dma_start(out=out[:, :], in_=red)

    return nc


def _pack_cols(vals, ntiles, pad):
    """[n] values -> [128, ntiles] f32 (column i holds positions 128i..128i+127)."""
    full = np.full(ntiles * 128, pad, dtype=np.float32)
    full[:len(vals)] = vals
    return np.ascontiguousarray(full.reshape(ntiles, 128).T)


_nc_cache = None
_nc_cache_key = None
last_results = None


def kernel(emissions, tags, mask, transitions, _trace=False):
    global _nc_cache, _nc_cache_key, last_results
    from concourse.bass_utils import run_bass_kernel_spmd

    em_all = np.asarray(emissions, dtype=np.float32).reshape(B * S, T)
    tg_all = np.asarray(tags).reshape(B, S).astype(np.int32)
    mk_all = np.asarray(mask).reshape(B, S).astype(bool)
    trf = np.ascontiguousarray(np.asarray(transitions, dtype=np.float32))

    cores = []
    for c in range(NCORES):
        r0 = c * BSH
        m = mk_all[r0:r0 + BSH]
        tg = tg_all[r0:r0 + BSH]
        idx_e = np.flatnonzero(m.reshape(-1))
        tag_e = tg.reshape(-1)[idx_e].astype(np.float32)
        # integer histogram of valid (prev,cur) tag pairs — index data only
        pmat = m[:, 1:] & m[:, :-1]
        bb, ss = np.nonzero(pmat)
        cmat = np.zeros((T, T), dtype=np.float32)
        np.add.at(cmat, (tg[bb, ss], tg[bb, ss + 1]), 1.0)
        cores.append((r0, idx_e, tag_e, cmat))

    def tiles_for(n):
        t = (n + 127) // 128
        return max(4, (t + 3) // 4 * 4)

    nte = max(tiles_for(len(c[1])) for c in cores)

    if _nc_cache_key != nte:
        _nc_cache = _build(nte)
        _nc_cache_key = nte
    nc = _nc_cache

    in_maps = []
    for (r0, idx_e, tag_e, cmat) in cores:
        n_e = len(idx_e)
        emp = np.zeros((nte * 128, T), dtype=BF16)
        emp[:n_e] = em_all[r0 * S + idx_e].astype(BF16)
        # SBUF image: partition p, tile i  <-> stream position 128*i + p
        em_img = np.ascontiguousarray(
            emp.reshape(nte, 128, T).transpose(1, 0, 2).reshape(128, nte * T))
        in_maps.append({
            'em': em_img,
            'tge': _pack_cols(tag_e, nte, PAD_TAG),
            'cm': cmat,
            'tr': trf,
        })

    res = run_bass_kernel_spmd(nc, in_maps, core_ids=list(range(NCORES)),
                               trace=_trace)
    last_results = res
    emit = trans = 0.0
    for r in res.results:
        v = np.asarray(r['out'], dtype=np.float64)
        emit += v[:, 0].sum()
        trans += v[:, 1].sum()
    cnt = float(mk_all.sum())
    return np.float32((emit + trans) / cnt)


# revision 26
# speedup vs baseline: 1.0829x; 1.0829x over previous
"""CRF loss kernel for Trainium2 (8 NeuronCores, data-parallel over batch).

v3 strategy — compacted emission stream + host-side pair histogram:
  - The reference loss = (sum_m E[b,s,tags] + sum_pm Tr[tag_prev,tag_cur]) / sum(m).
    All float arithmetic on emissions / transitions happens ON DEVICE; the
    host only marshals indices (mask compaction, tag packing, and the
    integer pair-count histogram C — all derived purely from tags+mask).
  - Emission score: host compacts the ~50% unmasked positions into a
    bf16 emission stream laid out as the exact SBUF image.  Per
    128-position tile the device builds a one-hot of the tags and
    accumulates Hm^T @ E into a [T,T] PSUM; the diagonal sums to the
    emission score.  One-hot builds are load-balanced over DVE
    (tensor_scalar is_equal in 4x mode, ~94ns), GPSIMD (~273ns) and ACT
    (relu(1-|tag-iota|), 2 activations, ~580ns).
  - Transition score: C[t1,t2] = #(valid pairs with tags t1->t2) is an
    integer histogram of index data, computed host-side; the device
    computes sum(C * transitions) with a fused multiply-reduce.
  - The emission stream lives wholly in SBUF (~34 KiB/partition), loaded
    with 4 large chunked DMAs (>=512B contiguous per descriptor, issued
    after the small tag/C/Tr tables so builds start immediately).
  - Per-core output is [128,2] per-partition partial sums; the 8-way
    combine, the mask count and the division happen host-side.
"""
import sys
import json

for p in ('/opt/trn_rl_repo', '/opt/trn_rl_repo/concourse'):
    if p not in sys.path:
        sys.path.insert(0, p)

import numpy as np
import ml_dtypes

BF16 = ml_dtypes.bfloat16
B, S, T = 512, 512, 128
NCORES = 8
BSH = B // NCORES              # 64 batch rows per core
NPOS = BSH * S                 # 32768 positions per core
PAD_TAG = 200.0                # out of [0,T) -> one-hot row is all zero


def _split_waits_json(bir_bytes: bytes, max_waits: int = 1) -> bytes:
    """This walrus build accepts at most ONE sync-wait per instruction;
    hoist extra waits onto single-wait NoOps inserted before the inst."""
    d = json.loads(bir_bytes)
    ctr = 0
    for f in d['functions']:
        for blk in f['blocks']:
            insts = blk.get('instructions')
            if not insts:
                continue
            out = []
            changed = False
            for ins in insts:
                si = ins.get('sync_info')
                if si and len(si.get('on_wait') or []) > max_waits:
                    waits = si['on_wait']
                    for w in waits[:-max_waits]:
                        ctr += 1
                        nop = {'engine': ins['engine'], 'ins': [], 'outs': [],
                               'name': f'wsplit-{ctr}', 'opcode': 'NoOp',
                               'sync_info': {'on_wait': [w], 'on_update': []}}
                        if 'debug' in ins:
                            nop['debug'] = ins['debug']
                        out.append(nop)
                    si['on_wait'] = waits[-max_waits:]
                    changed = True
                out.append(ins)
            if changed:
                blk['instructions'] = out
    return json.dumps(d).encode()


_patched = False


def _install_patch(bass_module):
    global _patched
    if _patched:
        return
    _patched = True
    orig = bass_module.Bass.to_json_bytes

    def patched(self):
        return _split_waits_json(orig(self))

    bass_module.Bass.to_json_bytes = patched


def _build(nte):
    """nte: number of 128-position tiles in the emission stream (mult of 4)."""
    import concourse.bass as bass
    import concourse.mybir as mybir
    import concourse.tile as tile
    from concourse.masks import make_identity
    _install_patch(bass)
    f32 = mybir.dt.float32
    bf16 = mybir.dt.bfloat16
    i32 = mybir.dt.int32
    Alu = mybir.AluOpType
    Af = mybir.ActivationFunctionType

    nbe = nte // 4                 # emission blocks of [128, 4, 128]

    nc = bass.Bass()
    em = nc.dram_tensor('em', [128, nte * T], bf16, kind='ExternalInput')
    tge = nc.dram_tensor('tge', [128, nte], f32, kind='ExternalInput')
    cm = nc.dram_tensor('cm', [T, T], f32, kind='ExternalInput')
    tr = nc.dram_tensor('tr', [T, T], f32, kind='ExternalInput')
    out = nc.dram_tensor('out', [128, 2], f32, kind='ExternalOutput')

    # ~6-block chunks so PE consumption keeps pace with transfers, with a
    # 1-block final chunk so the post-last-transfer tail is minimal
    chunks = []
    if nbe > 1:
        body = nbe - 1
        n_chunks = max(1, (body + 5) // 6)
        per_chunk = (body + n_chunks - 1) // n_chunks
        g0 = 0
        while g0 < body:
            g1 = min(body, g0 + per_chunk)
            chunks.append((g0, g1))
            g0 = g1
    chunks.append((nbe - 1, nbe))

    with tile.TileContext(nc) as tc:
        with tc.tile_pool(name='per', bufs=1) as per, \
             tc.tile_pool(name='hblk', bufs=34) as hblk, \
             tc.tile_pool(name='ps', bufs=1, space='PSUM') as psp:

            # ---- DMA order: first emission chunk, tag table, remaining
            # chunks, then the epilogue-only C/Tr tables.  The single
            # DMA-engines resource serializes transfers in this order. ----
            em_chunks = []

            def load_chunk(g0, g1):
                et = per.tile([128, (g1 - g0) * 4 * T], bf16, tag=f'emc{g0}')
                nc.sync.dma_start(out=et, in_=em[:, g0 * 4 * T:g1 * 4 * T])
                em_chunks.append((g0, g1, et))

            load_chunk(*chunks[0])
            tge_sb = per.tile([128, nte], f32)
            nc.sync.dma_start(out=tge_sb, in_=tge[:, :])
            for (g0, g1) in chunks[1:]:
                load_chunk(g0, g1)
            cm_sb = per.tile([128, 128], f32)
            nc.sync.dma_start(out=cm_sb, in_=cm[:, :])
            t_sb = per.tile([128, 128], f32)
            nc.sync.dma_start(out=t_sb, in_=tr[:, :])

            # ---- constants ----
            iota_i = per.tile([128, 128], i32)
            nc.gpsimd.iota(iota_i, pattern=[[1, 128]], base=0, channel_multiplier=0)
            iota_b = per.tile([128, 128], bf16)
            nc.vector.tensor_copy(iota_b, iota_i)
            ident = per.tile([128, 128], f32)
            make_identity(nc, ident)

            ps_emit = psp.tile([128, 128], f32)
            red = per.tile([128, 2], f32)
            nc.vector.memset(red, 0.0)

            # ---- one-hot builds: greedy balance over DVE/GPSIMD/ACT ----
            load = {'d': 0.0, 'p': 0.0, 'a': 0.0}
            COST = {'d': 376.0, 'p': 1092.0, 'a': 2330.0}

            def build_tile4(dst, col_tile, k0):
                e = min(load, key=lambda x: load[x] + COST[x])
                load[e] += COST[e]
                if e == 'a':
                    tmp = hblk.tile([128, 4, 128], bf16, tag='atmp')
                    for j in range(4):
                        nc.scalar.activation(tmp[:, j, :], iota_b, Af.Abs,
                                             bias=col_tile[:, k0 + j:k0 + j + 1],
                                             scale=-1.0)
                        nc.scalar.activation(dst[:, j, :], tmp[:, j, :], Af.Relu,
                                             bias=1.0, scale=-1.0)
                    return
                eng = nc.vector if e == 'd' else nc.gpsimd
                for j in range(4):
                    eng.tensor_scalar(out=dst[:, j, :], in0=iota_b,
                                      scalar1=col_tile[:, k0 + j:k0 + j + 1],
                                      scalar2=None, op0=Alu.is_equal)

            n_mm = [0]
            for (g0, g1, et) in em_chunks:
                for g in range(g0, g1):
                    hm = hblk.tile([128, 4, 128], bf16, tag='hm')
                    build_tile4(hm, tge_sb, 4 * g)
                    for j in range(4):
                        first = n_mm[0] == 0
                        n_mm[0] += 1
                        last = n_mm[0] == nte
                        col = ((g - g0) * 4 + j) * T
                        nc.tensor.matmul(ps_emit, lhsT=hm[:, j, :],
                                         rhs=et[:, col:col + T],
                                         start=first, stop=last,
                                         skip_group_check=True)

            # ---- partial sums, one fused multiply+row-sum each: the
            # transition product on GPSIMD (idle, inputs ready mid-kernel),
            # the emission diag on DVE right after the last matmul ----
            scr2 = per.tile([128, 128], f32)
            nc.vector.scalar_tensor_tensor(out=scr2, in0=cm_sb, scalar=1.0,
                                           in1=t_sb, op0=Alu.mult, op1=Alu.mult,
                                           accum_out=red[:, 1:2])
            scr = per.tile([128, 128], f32)
            nc.vector.scalar_tensor_tensor(out=scr, in0=ps_emit, scalar=1.0,
                                           in1=ident, op0=Alu.mult, op1=Alu.mult,
                                           accum_out=red[:, 0:1])
            nc.sync.dma_start(out=out[:, :], in_=red)

    return nc


def _pack_cols(vals, ntiles, pad):
    """[n] values -> [128, ntiles] f32 (column i holds positions 128i..128i+127)."""
    full = np.full(ntiles * 128, pad, dtype=np.float32)
    full[:len(vals)] = vals
    return np.ascontiguousarray(full.reshape(ntiles, 128).T)


_nc_cache = None
_nc_cache_key = None
last_results = None


def kernel(emissions, tags, mask, transitions, _trace=False):
    global _nc_cache, _nc_cache_key, last_results
    from concourse.bass_utils import run_bass_kernel_spmd

    em_all = np.asarray(emissions, dtype=np.float32).reshape(B * S, T)
    tg_all = np.asarray(tags).reshape(B, S).astype(np.int32)
    mk_all = np.asarray(mask).reshape(B, S).astype(bool)
    trf = np.ascontiguousarray(np.asarray(transitions, dtype=np.float32))

    cores = []
    for c in range(NCORES):
        r0 = c * BSH
        m = mk_all[r0:r0 + BSH]
        tg = tg_all[r0:r0 + BSH]
        idx_e = np.flatnonzero(m.reshape(-1))
        tag_e = tg.reshape(-1)[idx_e].astype(np.float32)
        # integer histogram of valid (prev,cur) tag pairs — index data only
        pmat = m[:, 1:] & m[:, :-1]
        bb, ss = np.nonzero(pmat)
        cmat = np.zeros((T, T), dtype=np.float32)
        np.add.at(cmat, (tg[bb, ss], tg[bb, ss + 1]), 1.0)
        cores.append((r0, idx_e, tag_e, cmat))

    def tiles_for(n):
        t = (n + 127) // 128
        return max(4, (t + 3) // 4 * 4)

    nte = max(tiles_for(len(c[1])) for c in cores)

    if _nc_cache_key != nte:
        _nc_cache = _build(nte)
        _nc_cache_key = nte
    nc = _nc_cache

    in_maps = []
    for (r0, idx_e, tag_e, cmat) in cores:
        n_e = len(idx_e)
        emp = np.zeros((nte * 128, T), dtype=BF16)
        emp[:n_e] = em_all[r0 * S + idx_e].astype(BF16)
        # SBUF image: partition p, tile i  <-> stream position 128*i + p
        em_img = np.ascontiguousarray(
            emp.reshape(nte, 128, T).transpose(1, 0, 2).reshape(128, nte * T))
        in_maps.append({
            'em': em_img,
            'tge': _pack_cols(tag_e, nte, PAD_TAG),
            'cm': cmat,
            'tr': trf,
        })

    res = run_bass_kernel_spmd(nc, in_maps, core_ids=list(range(NCORES)),
                               trace=_trace)
    last_results = res
    emit = trans = 0.0
    for r in res.results:
        v = np.asarray(r['out'], dtype=np.float64)
        emit += v[:, 0].sum()
        trans += v[:, 1].sum()
    cnt = float(mk_all.sum())
    return np.float32((emit + trans) / cnt)
